# revision 13
# baseline (speedup 1.0000x reference)
"""Trainium2 Bass kernel for segment-reduce classifier.

Reference computation:
    local = relu(x @ Wloc.T)            # [L, 128]
    feats = local.reshape(-1, 30, 128).mean(1)   # [L/30, 128]
    out   = feats @ W.T                 # [L/30, 10]

Strategy (8 NeuronCores, data-parallel on rows):
  - Each core gets R = L/8 = 150000 rows, host-transposed, fp16-cast, packed
    as xt [128, 75000]: partitions 0-63 = x_shard[:75000].T ("A" half),
    partitions 64-127 = x_shard[75000:].T ("B" half).  Rows are additionally
    permuted j-major within each 480-row chunk on the host (col = j*16+g for
    row g*30+j) so every on-chip access pattern has contiguous inner runs.
  - matmul1 (fp16, 1 cyc/row): lhsT = Wloc.T stacked twice [128, 128]; two
    concurrent K=64 matmuls via PE row-groups produce localT [128enc, rows]
    in 480-row chunk pairs (A+B) in 2-bank PSUM tiles.
  - relu PSUM -> SBUF fp16 is the kernel bottleneck: every element crosses
    at ~1 elem/cyc/partition on ACT or DVE (GpSimd cannot access PSUM on
    TRN2).  Each 2-bank PSUM tile (960 elems/partition) drains in ONE
    instruction, greedy-balanced between ACT and DVE by modeled cost;
    3 PSUM bufs keep one fill + two drains in flight.
  - mean-pool + classifier fused: accumulating matmuls per tile (one per
    within-segment offset j; rhs g-runs contiguous thanks to the j-major
    permutation) -> pooling is free PSUM accumulation. M=10 is packed 4x
    into PE column-groups (tile_position (0,32s)); each strip accumulates
    ~8 of the 30 j's and the 4 strips are summed on the host.  The previous
    tile's classifier matmuls are INTERLEAVED between mm1 pairs in emission
    order so the PE never starves the relu pipeline.  The PE ifmap port is
    the hard wall: mm1 streams 75000 cols + 20000 weight-reload cols, mm2
    streams all 150000 rl cols once (K=128, no row-group trick possible).
  - a short burst of dummy matmuls at kernel start keeps the PE busy during
    the first DMA so the p-state ramps to 2.4 GHz early.
  - acc PSUM -> SBUF drain casts to fp16 (engine-balanced like the relu);
    per-tile DMA out (fp16); host sums the 4 column strips and reorders.
"""

import numpy as np

import concourse.bacc as bacc
import concourse.bass as bass
import concourse.tile as tile
from concourse import mybir
from concourse.bass_utils import run_bass_kernel_spmd

# Problem constants (hardcoded per harness contract)
L, D_IN, D_ENC, C, J = 1200000, 64, 128, 10, 30
N_CORES = 8
R = L // N_CORES          # rows per core = 150000
HALF = R // 2             # 75000 cols per half-stream
CH = 480                  # chunk rows (16 segments) per matmul slot
# first DMA tile split small so the pipeline starts early
TFS = [1920, 5760] + [7680] * 8 + [5880]   # sum = 75000
SEG_PER_CORE = R // J     # 5000
# j-subsets for the 4 PE column-group strips of the classifier matmul
J_SETS = [list(range(0, 8)), list(range(8, 16)),
          list(range(16, 23)), list(range(23, 30))]

# measured per-element / per-instruction engine costs (ns) for balancing
ENG_COST = {
    "A": (0.911, 185.0),   # ACT: measured 1060ns @ 960 elems
    "D": (1.075, 125.0),   # DVE: measured 1157ns @ 960 elems
}

_CACHE = {}


def _build_kernel():
    nc = bacc.Bacc("TRN2", target_bir_lowering=False, debug=False,
                   num_devices=N_CORES)
    f32, f16 = mybir.dt.float32, mybir.dt.float16

    xt_d = nc.dram_tensor("xt", [128, HALF], f16, kind="ExternalInput")
    w1_d = nc.dram_tensor("w1", [128, D_ENC], f16, kind="ExternalInput")
    w2_d = nc.dram_tensor("w2", [128, C], f16, kind="ExternalInput")
    out_d = nc.dram_tensor("out", [128, SEG_PER_CORE], f16,
                           kind="ExternalOutput")

    load = {"A": 0.0, "D": 0.0}

    def pick(n):
        e = min(load, key=lambda k: load[k] + ENG_COST[k][0] * n
                + ENG_COST[k][1])
        load[e] += ENG_COST[e][0] * n + ENG_COST[e][1]
        return e

    def emit_relu(rout, pin, n):
        e = pick(n)
        if e == "A":
            nc.scalar.activation(rout, pin,
                                 mybir.ActivationFunctionType.Relu)
        else:
            nc.vector.tensor_scalar_max(rout, pin, 0.0)

    with tile.TileContext(nc) as tc:
        with (
            tc.tile_pool(name="consts", bufs=1) as consts,
            tc.tile_pool(name="xin", bufs=3) as xin,
            tc.tile_pool(name="rlp", bufs=3) as rlp,
            tc.tile_pool(name="outp", bufs=2) as outp,
            tc.tile_pool(name="psp", bufs=3, space="PSUM") as psp,
            tc.tile_pool(name="accp", bufs=2, space="PSUM") as accp,
        ):
            w1 = consts.tile([128, D_ENC], f16)
            nc.sync.dma_start(w1[:], w1_d[:])
            w2 = consts.tile([128, C], f16)
            nc.sync.dma_start(w2[:], w2_d[:])

            # PE warmup: keep the tensor engine streaming during the first
            # xt DMA so the p-state ramps to full clock before real work
            dum = consts.tile([128, 512], f16)
            nc.gpsimd.memset(dum[:], 0)
            wacc = accp.tile([128, 512], f32, tag="acc", name="warm")
            for _ in range(5):
                nc.tensor.matmul(wacc[:], dum[:, 0:128], dum[:],
                                 start=True, stop=True)

            def mm2_rounds(tf, rl, ocol):
                """Generator: classifier+pool matmul rounds for one tile,
                then the acc drain + out DMA.  rl slot-major layout:
                flat col = c*960 + h*480 + j*gc + g."""
                gt = tf // J
                acc = accp.tile([128, 512], f32, tag="acc", name="acc")
                acv = acc.rearrange("p (h g) -> p h g", h=2)  # h-stride 256
                nfull = tf // CH
                rem = tf % CH
                gfull = nfull * (CH // J)
                rfull_all = rl[:, 0:nfull * 2 * CH].rearrange(
                    "p (c h j g) -> p h c j g", c=nfull, h=2, j=J)
                if rem:
                    rrem_all = rl[:, nfull * 2 * CH:nfull * 2 * CH + 2 * rem
                                  ].rearrange("p (h j g) -> p h j g",
                                              h=2, j=J)
                for k in range(8):
                    for s in range(4):
                        if k >= len(J_SETS[s]):
                            continue
                        j = J_SETS[s][k]
                        first, last = k == 0, k == len(J_SETS[s]) - 1
                        aout = acv[32 * s:32 * s + C, :, 0:gfull]
                        nc.tensor.matmul(aout, w2[:], rfull_all[:, :, :, j, :],
                                         start=first,
                                         stop=(last and rem == 0),
                                         tile_position=(0, 32 * s))
                        if rem:
                            arem = acv[32 * s:32 * s + C, :,
                                       gfull:gfull + rem // J]
                            nc.tensor.matmul(arem, w2[:],
                                             rrem_all[:, :, j, :],
                                             start=False, stop=last,
                                             tile_position=(0, 32 * s))
                        if s % 2 == 1:
                            yield
                # drain accum -> staging fp16 (engine-balanced), DMA out
                av = acc.rearrange("p (h g) -> p h g", h=2)[:, :, 0:gt]
                ob = outp.tile([128, 512], f16, tag="ob")
                ov = ob[:, 0:2 * gt].rearrange("p (h g) -> p h g", h=2)
                e = pick(2 * gt)
                if e == "A":
                    nc.scalar.copy(ov, av)
                else:
                    nc.vector.tensor_copy(ov, av)
                nc.sync.dma_start(out_d[:, ocol:ocol + 2 * gt],
                                  ob[:, 0:2 * gt])
                yield

            ocol = 0
            col0 = 0
            pending = None   # mm2 generator of the previous tile

            for t, tf in enumerate(TFS):
                gt = tf // J
                # ---- load xt tile [128, tf] fp16 (contiguous) ----
                xt = xin.tile([128, 7680], f16, tag="xt")
                nc.sync.dma_start(xt[:, 0:tf], xt_d[:, col0:col0 + tf])

                # relu output, slot-major: chunk c A at 960c, B at 960c+480
                rl = rlp.tile([128, 2 * 7680], f16, tag="rl")

                chunks = [CH] * (tf // CH) + ([tf % CH] if tf % CH else [])
                cb = 0
                for ci, ch in enumerate(chunks):
                    # PSUM pair tile: bank0 = A chunk, bank1 = B chunk
                    pp = psp.tile([128, 1024], f32, tag="pp")
                    nc.tensor.matmul(pp[:, 0:ch], w1[0:64, :],
                                     xt[0:64, cb:cb + ch])
                    nc.tensor.matmul(pp[:, 512:512 + ch], w1[64:128, :],
                                     xt[64:128, cb:cb + ch])
                    # interleave classifier matmuls of the previous tile
                    # between mm1 pairs (16 yields per tile)
                    if pending is not None:
                        next(pending, None)

                    # drain whole pp tile in one instruction (contiguous out)
                    pin = pp.rearrange("p (s q) -> p s q", s=2)[:, :, 0:ch]
                    rout = rl[:, 2 * cb:2 * cb + 2 * ch].rearrange(
                        "p (s q) -> p s q", s=2)
                    emit_relu(rout, pin, 2 * ch)
                    cb += ch

                # flush any remaining rounds of the previous tile
                if pending is not None:
                    for _ in pending:
                        pass
                pending = mm2_rounds(tf, rl, ocol)
                col0 += tf
                ocol += 2 * gt
            for _ in pending:
                pass

    nc.compile()
    return nc


def _jmajor_index():
    """Column permutation for one 75000-row half: within each 480-row chunk
    (16 segments) store rows j-major (col j*gc+g <- row g*30+j)."""
    idx = np.empty(HALF, dtype=np.int64)
    b = 0
    sizes = []
    for tf in TFS:
        sizes += [CH] * (tf // CH) + ([tf % CH] if tf % CH else [])
    for ch in sizes:
        gc = ch // J
        m = np.arange(ch).reshape(gc, J)   # m[g, j] = g*30 + j
        idx[b:b + ch] = b + m.T.ravel()    # packed[j*gc+g] = g*30+j
        b += ch
    assert b == HALF
    return idx


def kernel(x: np.ndarray, Wloc: np.ndarray, W: np.ndarray) -> np.ndarray:
    if "nc" not in _CACHE:
        _CACHE["nc"] = _build_kernel()
        _CACHE["idx"] = _jmajor_index()
    nc = _CACHE["nc"]
    idx = _CACHE["idx"]

    x = np.asarray(x, dtype=np.float32)
    # pack per-core transposed fp16 inputs: [8, 128, HALF], j-major chunks
    xp = x.reshape(N_CORES, 2, HALF, D_IN)[:, :, idx, :].transpose(0, 1, 3, 2)
    xp = np.ascontiguousarray(xp, dtype=np.float16).reshape(N_CORES, 128, HALF)

    w1 = np.ascontiguousarray(
        np.concatenate([Wloc.T, Wloc.T], axis=0), dtype=np.float16)  # [128,128]
    w2 = np.ascontiguousarray((W / float(J)).T, dtype=np.float16)    # [128,10]

    in_maps = [{"xt": xp[c], "w1": w1, "w2": w2} for c in range(N_CORES)]
    res = run_bass_kernel_spmd(nc, in_maps, core_ids=list(range(N_CORES)))
    _CACHE["exec_time_ns"] = res.exec_time_ns
    _CACHE["trace"] = res.instructions_and_trace

    # host: sum the 4 PE column-group strips, then reorder segments
    out = np.empty((L // J, C), dtype=np.float32)
    for c in range(N_CORES):
        oc = res.results[c]["out"].astype(np.float32)  # [128, 5000] fp16
        strips = oc[0:10] + oc[32:42] + oc[64:74] + oc[96:106]  # [10, 5000]
        ocol = 0
        gbase = 0
        base = c * SEG_PER_CORE
        for tf in TFS:
            gt = tf // J
            blk = strips[:, ocol:ocol + 2 * gt].reshape(C, 2, gt)
            out[base + gbase:base + gbase + gt] = blk[:, 0].T
            out[base + HALF // J + gbase:base + HALF // J + gbase + gt] = blk[:, 1].T
            ocol += 2 * gt
            gbase += gt
    return out


# revision 15
# speedup vs baseline: 1.0685x; 1.0685x over previous
"""Trainium2 Bass kernel for segment-reduce classifier.

Reference computation:
    local = relu(x @ Wloc.T)            # [L, 128]
    feats = local.reshape(-1, 30, 128).mean(1)   # [L/30, 128]
    out   = feats @ W.T                 # [L/30, 10]

Strategy (8 NeuronCores, data-parallel on rows):
  - Each core gets R = L/8 = 150000 rows, host-transposed, fp16-cast, packed
    as xt [128, 75000]: partitions 0-63 = x_shard[:75000].T ("A" half),
    partitions 64-127 = x_shard[75000:].T ("B" half).  Rows are additionally
    permuted j-major within each 480-row chunk on the host (col = j*16+g for
    row g*30+j) so every on-chip access pattern has contiguous inner runs.
  - matmul1 (fp16, 1 cyc/row): lhsT = Wloc.T stacked twice [128, 128]; two
    concurrent K=64 matmuls via PE row-groups produce localT [128enc, rows]
    in 480-row chunk pairs (A+B) in 2-bank PSUM tiles.
  - relu PSUM -> SBUF fp16 is the kernel bottleneck: every element crosses
    at ~1 elem/cyc/partition on ACT or DVE (GpSimd cannot access PSUM on
    TRN2).  Each 2-bank PSUM tile (960 elems/partition) drains in ONE
    instruction, greedy-balanced between ACT and DVE by modeled cost;
    3 PSUM bufs keep one fill + two drains in flight.
  - mean-pool + classifier fused: accumulating matmuls per tile (one per
    within-segment offset j; rhs g-runs contiguous thanks to the j-major
    permutation) -> pooling is free PSUM accumulation. M=10 is packed 4x
    into PE column-groups (tile_position (0,32s)); each strip accumulates
    ~8 of the 30 j's and the 4 strips are summed on the host.  The previous
    tile's classifier matmuls are INTERLEAVED between mm1 pairs in emission
    order so the PE never starves the relu pipeline.  The PE ifmap port is
    the hard wall: mm1 streams 75000 cols + 20000 weight-reload cols, mm2
    streams all 150000 rl cols once (K=128, no row-group trick possible).
  - a short burst of dummy matmuls at kernel start keeps the PE busy during
    the first DMA so the p-state ramps to 2.4 GHz early.
  - acc PSUM -> SBUF drain casts to fp16 (engine-balanced like the relu);
    per-tile DMA out (fp16); host sums the 4 column strips and reorders.
"""

import numpy as np

import concourse.bacc as bacc
import concourse.bass as bass
import concourse.tile as tile
from concourse import mybir
from concourse.bass_utils import run_bass_kernel_spmd

# Problem constants (hardcoded per harness contract)
L, D_IN, D_ENC, C, J = 1200000, 64, 128, 10, 30
N_CORES = 8
R = L // N_CORES          # rows per core = 150000
HALF = R // 2             # 75000 cols per half-stream
CH = 480                  # chunk rows (16 segments) per matmul slot
# first DMA tile split small so the pipeline starts early
TFS = [1920, 5760] + [7680] * 8 + [5880]   # sum = 75000
SEG_PER_CORE = R // J     # 5000
# j-subsets for the 4 PE column-group strips of the classifier matmul
J_SETS = [list(range(0, 8)), list(range(8, 16)),
          list(range(16, 23)), list(range(23, 30))]

# measured per-element / per-instruction engine costs (ns) for balancing
ENG_COST = {
    "A": (0.911, 185.0),   # ACT: measured 1060ns @ 960 elems
    "D": (1.075, 125.0),   # DVE: measured 1157ns @ 960 elems
}

_CACHE = {}


def _build_kernel():
    nc = bacc.Bacc("TRN2", target_bir_lowering=False, debug=False,
                   num_devices=N_CORES)
    f32, f16 = mybir.dt.float32, mybir.dt.float16

    xt_d = nc.dram_tensor("xt", [128, HALF], f16, kind="ExternalInput")
    w1_d = nc.dram_tensor("w1", [128, D_ENC], f16, kind="ExternalInput")
    w2_d = nc.dram_tensor("w2", [128, C], f16, kind="ExternalInput")
    out_d = nc.dram_tensor("out", [128, SEG_PER_CORE], f16,
                           kind="ExternalOutput")

    load = {"A": 0.0, "D": 0.0}

    def pick(n):
        e = min(load, key=lambda k: load[k] + ENG_COST[k][0] * n
                + ENG_COST[k][1])
        load[e] += ENG_COST[e][0] * n + ENG_COST[e][1]
        return e

    def emit_relu(rout, pin, n):
        e = pick(n)
        if e == "A":
            nc.scalar.activation(rout, pin,
                                 mybir.ActivationFunctionType.Relu)
        else:
            nc.vector.tensor_scalar_max(rout, pin, 0.0)

    with tile.TileContext(nc) as tc:
        with (
            tc.tile_pool(name="consts", bufs=1) as consts,
            tc.tile_pool(name="xin", bufs=3) as xin,
            tc.tile_pool(name="rlp", bufs=3) as rlp,
            tc.tile_pool(name="outp", bufs=2) as outp,
            tc.tile_pool(name="psp", bufs=3, space="PSUM") as psp,
            tc.tile_pool(name="accp", bufs=2, space="PSUM") as accp,
        ):
            w1 = consts.tile([128, D_ENC], f16)
            nc.sync.dma_start(w1[:], w1_d[:])
            w2 = consts.tile([128, C], f16)
            nc.sync.dma_start(w2[:], w2_d[:])

            # PE warmup: keep the tensor engine streaming during the first
            # xt DMA so the p-state ramps to full clock before real work
            dum = consts.tile([128, 512], f16)
            nc.gpsimd.memset(dum[:], 0)
            wacc = accp.tile([128, 512], f32, tag="acc", name="warm")
            for _ in range(5):
                nc.tensor.matmul(wacc[:], dum[:, 0:128], dum[:],
                                 start=True, stop=True)

            def mm2_rounds(tf, rl, ocol):
                """Generator: classifier+pool matmul rounds for one tile,
                then the acc drain + out DMA.  rl slot-major layout:
                flat col = c*960 + h*480 + j*gc + g."""
                gt = tf // J
                acc = accp.tile([128, 512], f32, tag="acc", name="acc")
                acv = acc.rearrange("p (h g) -> p h g", h=2)  # h-stride 256
                nfull = tf // CH
                rem = tf % CH
                gfull = nfull * (CH // J)
                rfull_all = rl[:, 0:nfull * 2 * CH].rearrange(
                    "p (c h j g) -> p h c j g", c=nfull, h=2, j=J)
                if rem:
                    rrem_all = rl[:, nfull * 2 * CH:nfull * 2 * CH + 2 * rem
                                  ].rearrange("p (h j g) -> p h j g",
                                              h=2, j=J)
                for k in range(8):
                    for s in range(4):
                        if k >= len(J_SETS[s]):
                            continue
                        j = J_SETS[s][k]
                        first, last = k == 0, k == len(J_SETS[s]) - 1
                        aout = acv[32 * s:32 * s + C, :, 0:gfull]
                        nc.tensor.matmul(aout, w2[:], rfull_all[:, :, :, j, :],
                                         start=first,
                                         stop=(last and rem == 0),
                                         tile_position=(0, 32 * s))
                        if rem:
                            arem = acv[32 * s:32 * s + C, :,
                                       gfull:gfull + rem // J]
                            nc.tensor.matmul(arem, w2[:],
                                             rrem_all[:, :, j, :],
                                             start=False, stop=last,
                                             tile_position=(0, 32 * s))
                    yield
                # drain accum -> staging fp16 (engine-balanced), DMA out
                av = acc.rearrange("p (h g) -> p h g", h=2)[:, :, 0:gt]
                ob = outp.tile([128, 512], f16, tag="ob")
                ov = ob[:, 0:2 * gt].rearrange("p (h g) -> p h g", h=2)
                e = pick(2 * gt)
                if e == "A":
                    nc.scalar.copy(ov, av)
                else:
                    nc.vector.tensor_copy(ov, av)
                nc.sync.dma_start(out_d[:, ocol:ocol + 2 * gt],
                                  ob[:, 0:2 * gt])
                yield

            ocol = 0
            col0 = 0
            pending = None   # mm2 generator of the previous tile

            for t, tf in enumerate(TFS):
                gt = tf // J
                # ---- load xt tile [128, tf] fp16 (contiguous) ----
                xt = xin.tile([128, 7680], f16, tag="xt")
                nc.sync.dma_start(xt[:, 0:tf], xt_d[:, col0:col0 + tf])

                # relu output, slot-major: chunk c A at 960c, B at 960c+480
                rl = rlp.tile([128, 2 * 7680], f16, tag="rl")

                chunks = [CH] * (tf // CH) + ([tf % CH] if tf % CH else [])
                cb = 0
                for ci, ch in enumerate(chunks):
                    # PSUM pair tile: bank0 = A chunk, bank1 = B chunk
                    pp = psp.tile([128, 1024], f32, tag="pp")
                    nc.tensor.matmul(pp[:, 0:ch], w1[0:64, :],
                                     xt[0:64, cb:cb + ch])
                    nc.tensor.matmul(pp[:, 512:512 + ch], w1[64:128, :],
                                     xt[64:128, cb:cb + ch])
                    # interleave one classifier round of the previous tile
                    # after every other mm1 pair
                    if pending is not None and ci % 2 == 1:
                        next(pending, None)

                    # drain whole pp tile in one instruction (contiguous out)
                    pin = pp.rearrange("p (s q) -> p s q", s=2)[:, :, 0:ch]
                    rout = rl[:, 2 * cb:2 * cb + 2 * ch].rearrange(
                        "p (s q) -> p s q", s=2)
                    emit_relu(rout, pin, 2 * ch)
                    cb += ch

                # flush any remaining rounds of the previous tile
                if pending is not None:
                    for _ in pending:
                        pass
                pending = mm2_rounds(tf, rl, ocol)
                col0 += tf
                ocol += 2 * gt
            for _ in pending:
                pass

    nc.compile()
    return nc


def _jmajor_index():
    """Column permutation for one 75000-row half: within each 480-row chunk
    (16 segments) store rows j-major (col j*gc+g <- row g*30+j)."""
    idx = np.empty(HALF, dtype=np.int64)
    b = 0
    sizes = []
    for tf in TFS:
        sizes += [CH] * (tf // CH) + ([tf % CH] if tf % CH else [])
    for ch in sizes:
        gc = ch // J
        m = np.arange(ch).reshape(gc, J)   # m[g, j] = g*30 + j
        idx[b:b + ch] = b + m.T.ravel()    # packed[j*gc+g] = g*30+j
        b += ch
    assert b == HALF
    return idx


def kernel(x: np.ndarray, Wloc: np.ndarray, W: np.ndarray) -> np.ndarray:
    if "nc" not in _CACHE:
        _CACHE["nc"] = _build_kernel()
        _CACHE["idx"] = _jmajor_index()
    nc = _CACHE["nc"]
    idx = _CACHE["idx"]

    x = np.asarray(x, dtype=np.float32)
    # pack per-core transposed fp16 inputs: [8, 128, HALF], j-major chunks
    xp = x.reshape(N_CORES, 2, HALF, D_IN)[:, :, idx, :].transpose(0, 1, 3, 2)
    xp = np.ascontiguousarray(xp, dtype=np.float16).reshape(N_CORES, 128, HALF)

    w1 = np.ascontiguousarray(
        np.concatenate([Wloc.T, Wloc.T], axis=0), dtype=np.float16)  # [128,128]
    w2 = np.ascontiguousarray((W / float(J)).T, dtype=np.float16)    # [128,10]

    in_maps = [{"xt": xp[c], "w1": w1, "w2": w2} for c in range(N_CORES)]
    res = run_bass_kernel_spmd(nc, in_maps, core_ids=list(range(N_CORES)))
    _CACHE["exec_time_ns"] = res.exec_time_ns
    _CACHE["trace"] = res.instructions_and_trace

    # host: sum the 4 PE column-group strips, then reorder segments
    out = np.empty((L // J, C), dtype=np.float32)
    for c in range(N_CORES):
        oc = res.results[c]["out"].astype(np.float32)  # [128, 5000] fp16
        strips = oc[0:10] + oc[32:42] + oc[64:74] + oc[96:106]  # [10, 5000]
        ocol = 0
        gbase = 0
        base = c * SEG_PER_CORE
        for tf in TFS:
            gt = tf // J
            blk = strips[:, ocol:ocol + 2 * gt].reshape(C, 2, gt)
            out[base + gbase:base + gbase + gt] = blk[:, 0].T
            out[base + HALF // J + gbase:base + HALF // J + gbase + gt] = blk[:, 1].T
            ocol += 2 * gt
            gbase += gt
    return out


# revision 16
# speedup vs baseline: 1.0732x; 1.0044x over previous
"""Trainium2 Bass kernel for segment-reduce classifier.

Reference computation:
    local = relu(x @ Wloc.T)            # [L, 128]
    feats = local.reshape(-1, 30, 128).mean(1)   # [L/30, 128]
    out   = feats @ W.T                 # [L/30, 10]

Strategy (8 NeuronCores, data-parallel on rows):
  - Each core gets R = L/8 = 150000 rows, host-transposed, fp16-cast, packed
    as xt [128, 75000]: partitions 0-63 = x_shard[:75000].T ("A" half),
    partitions 64-127 = x_shard[75000:].T ("B" half).  Rows are additionally
    permuted j-major within each 480-row chunk on the host (col = j*16+g for
    row g*30+j) so every on-chip access pattern has contiguous inner runs.
  - matmul1 (fp16, 1 cyc/row): lhsT = Wloc.T stacked twice [128, 128]; two
    concurrent K=64 matmuls via PE row-groups produce localT [128enc, rows]
    in 480-row chunk pairs (A+B) in 2-bank PSUM tiles.
  - relu PSUM -> SBUF fp16 is the kernel bottleneck: every element crosses
    at ~1 elem/cyc/partition on ACT or DVE (GpSimd cannot access PSUM on
    TRN2).  Each 2-bank PSUM tile (960 elems/partition) drains in ONE
    instruction, greedy-balanced between ACT and DVE by modeled cost;
    3 PSUM bufs keep one fill + two drains in flight.
  - mean-pool + classifier fused: accumulating matmuls per tile (one per
    within-segment offset j; rhs g-runs contiguous thanks to the j-major
    permutation) -> pooling is free PSUM accumulation. M=10 is packed 4x
    into PE column-groups (tile_position (0,32s)); each strip accumulates
    ~8 of the 30 j's and the 4 strips are summed on the host.  The previous
    tile's classifier matmuls are INTERLEAVED between mm1 pairs in emission
    order so the PE never starves the relu pipeline.  The PE ifmap port is
    the hard wall: mm1 streams 75000 cols + 20000 weight-reload cols, mm2
    streams all 150000 rl cols once (K=128, no row-group trick possible).
  - a short burst of dummy matmuls at kernel start keeps the PE busy during
    the first DMA so the p-state ramps to 2.4 GHz early.
  - acc PSUM -> SBUF drain casts to fp16 (engine-balanced like the relu);
    per-tile DMA out (fp16); host sums the 4 column strips and reorders.
"""

import numpy as np

import concourse.bacc as bacc
import concourse.bass as bass
import concourse.tile as tile
from concourse import mybir
from concourse.bass_utils import run_bass_kernel_spmd

# Problem constants (hardcoded per harness contract)
L, D_IN, D_ENC, C, J = 1200000, 64, 128, 10, 30
N_CORES = 8
R = L // N_CORES          # rows per core = 150000
HALF = R // 2             # 75000 cols per half-stream
CH = 480                  # chunk rows (16 segments) per matmul slot
# first DMA tile split small so the pipeline starts early; last tiles
# tapered so the final classifier flush has little work after the last relu
TFS = [1920, 5760] + [7680] * 8 + [3840, 1440, 600]   # sum = 75000
SEG_PER_CORE = R // J     # 5000
# j-subsets for the 4 PE column-group strips of the classifier matmul
J_SETS = [list(range(0, 8)), list(range(8, 16)),
          list(range(16, 23)), list(range(23, 30))]

# measured per-element / per-instruction engine costs (ns) for balancing
ENG_COST = {
    "A": (0.911, 185.0),   # ACT: measured 1060ns @ 960 elems
    "D": (1.075, 125.0),   # DVE: measured 1157ns @ 960 elems
}

_CACHE = {}


def _build_kernel():
    nc = bacc.Bacc("TRN2", target_bir_lowering=False, debug=False,
                   num_devices=N_CORES)
    f32, f16 = mybir.dt.float32, mybir.dt.float16

    xt_d = nc.dram_tensor("xt", [128, HALF], f16, kind="ExternalInput")
    w1_d = nc.dram_tensor("w1", [128, D_ENC], f16, kind="ExternalInput")
    w2_d = nc.dram_tensor("w2", [128, C], f16, kind="ExternalInput")
    out_d = nc.dram_tensor("out", [128, SEG_PER_CORE], f16,
                           kind="ExternalOutput")

    load = {"A": 0.0, "D": 0.0}

    def pick(n):
        e = min(load, key=lambda k: load[k] + ENG_COST[k][0] * n
                + ENG_COST[k][1])
        load[e] += ENG_COST[e][0] * n + ENG_COST[e][1]
        return e

    def emit_relu(rout, pin, n):
        e = pick(n)
        if e == "A":
            nc.scalar.activation(rout, pin,
                                 mybir.ActivationFunctionType.Relu)
        else:
            nc.vector.tensor_scalar_max(rout, pin, 0.0)

    with tile.TileContext(nc) as tc:
        with (
            tc.tile_pool(name="consts", bufs=1) as consts,
            tc.tile_pool(name="xin", bufs=3) as xin,
            tc.tile_pool(name="rlp", bufs=3) as rlp,
            tc.tile_pool(name="outp", bufs=2) as outp,
            tc.tile_pool(name="psp", bufs=3, space="PSUM") as psp,
            tc.tile_pool(name="accp", bufs=2, space="PSUM") as accp,
        ):
            w1 = consts.tile([128, D_ENC], f16)
            nc.sync.dma_start(w1[:], w1_d[:])
            w2 = consts.tile([128, C], f16)
            nc.sync.dma_start(w2[:], w2_d[:])

            # PE warmup: keep the tensor engine streaming during the first
            # xt DMA so the p-state ramps to full clock before real work
            dum = consts.tile([128, 512], f16)
            nc.gpsimd.memset(dum[:], 0)
            wacc = accp.tile([128, 512], f32, tag="acc", name="warm")
            for _ in range(5):
                nc.tensor.matmul(wacc[:], dum[:, 0:128], dum[:],
                                 start=True, stop=True)

            def mm2_rounds(tf, rl, ocol):
                """Generator: classifier+pool matmul rounds for one tile,
                then the acc drain + out DMA.  rl slot-major layout:
                flat col = c*960 + h*480 + j*gc + g."""
                gt = tf // J
                acc = accp.tile([128, 512], f32, tag="acc", name="acc")
                acv = acc.rearrange("p (h g) -> p h g", h=2)  # h-stride 256
                nfull = tf // CH
                rem = tf % CH
                gfull = nfull * (CH // J)
                rfull_all = rl[:, 0:nfull * 2 * CH].rearrange(
                    "p (c h j g) -> p h c j g", c=nfull, h=2, j=J)
                if rem:
                    rrem_all = rl[:, nfull * 2 * CH:nfull * 2 * CH + 2 * rem
                                  ].rearrange("p (h j g) -> p h j g",
                                              h=2, j=J)
                for k in range(8):
                    for s in range(4):
                        if k >= len(J_SETS[s]):
                            continue
                        j = J_SETS[s][k]
                        first, last = k == 0, k == len(J_SETS[s]) - 1
                        aout = acv[32 * s:32 * s + C, :, 0:gfull]
                        nc.tensor.matmul(aout, w2[:], rfull_all[:, :, :, j, :],
                                         start=first,
                                         stop=(last and rem == 0),
                                         tile_position=(0, 32 * s))
                        if rem:
                            arem = acv[32 * s:32 * s + C, :,
                                       gfull:gfull + rem // J]
                            nc.tensor.matmul(arem, w2[:],
                                             rrem_all[:, :, j, :],
                                             start=False, stop=last,
                                             tile_position=(0, 32 * s))
                    yield
                # drain accum -> staging fp16 (engine-balanced), DMA out
                av = acc.rearrange("p (h g) -> p h g", h=2)[:, :, 0:gt]
                ob = outp.tile([128, 512], f16, tag="ob")
                ov = ob[:, 0:2 * gt].rearrange("p (h g) -> p h g", h=2)
                e = pick(2 * gt)
                if e == "A":
                    nc.scalar.copy(ov, av)
                else:
                    nc.vector.tensor_copy(ov, av)
                nc.sync.dma_start(out_d[:, ocol:ocol + 2 * gt],
                                  ob[:, 0:2 * gt])
                yield

            ocol = 0
            col0 = 0
            pending = None   # mm2 generator of the previous tile

            for t, tf in enumerate(TFS):
                gt = tf // J
                # ---- load xt tile [128, tf] fp16 (contiguous) ----
                xt = xin.tile([128, 7680], f16, tag="xt")
                nc.sync.dma_start(xt[:, 0:tf], xt_d[:, col0:col0 + tf])

                # relu output, slot-major: chunk c A at 960c, B at 960c+480
                rl = rlp.tile([128, 2 * 7680], f16, tag="rl")

                chunks = [CH] * (tf // CH) + ([tf % CH] if tf % CH else [])
                cb = 0
                for ci, ch in enumerate(chunks):
                    # PSUM pair tile: bank0 = A chunk, bank1 = B chunk
                    pp = psp.tile([128, 1024], f32, tag="pp")
                    nc.tensor.matmul(pp[:, 0:ch], w1[0:64, :],
                                     xt[0:64, cb:cb + ch])
                    nc.tensor.matmul(pp[:, 512:512 + ch], w1[64:128, :],
                                     xt[64:128, cb:cb + ch])
                    # interleave one classifier round of the previous tile
                    # after every other mm1 pair
                    if pending is not None and ci % 2 == 1:
                        next(pending, None)

                    # drain whole pp tile in one instruction (contiguous out)
                    pin = pp.rearrange("p (s q) -> p s q", s=2)[:, :, 0:ch]
                    rout = rl[:, 2 * cb:2 * cb + 2 * ch].rearrange(
                        "p (s q) -> p s q", s=2)
                    emit_relu(rout, pin, 2 * ch)
                    cb += ch

                # flush any remaining rounds of the previous tile
                if pending is not None:
                    for _ in pending:
                        pass
                pending = mm2_rounds(tf, rl, ocol)
                col0 += tf
                ocol += 2 * gt
            for _ in pending:
                pass

    nc.compile()
    return nc


def _jmajor_index():
    """Column permutation for one 75000-row half: within each 480-row chunk
    (16 segments) store rows j-major (col j*gc+g <- row g*30+j)."""
    idx = np.empty(HALF, dtype=np.int64)
    b = 0
    sizes = []
    for tf in TFS:
        sizes += [CH] * (tf // CH) + ([tf % CH] if tf % CH else [])
    for ch in sizes:
        gc = ch // J
        m = np.arange(ch).reshape(gc, J)   # m[g, j] = g*30 + j
        idx[b:b + ch] = b + m.T.ravel()    # packed[j*gc+g] = g*30+j
        b += ch
    assert b == HALF
    return idx


def kernel(x: np.ndarray, Wloc: np.ndarray, W: np.ndarray) -> np.ndarray:
    if "nc" not in _CACHE:
        _CACHE["nc"] = _build_kernel()
        _CACHE["idx"] = _jmajor_index()
    nc = _CACHE["nc"]
    idx = _CACHE["idx"]

    x = np.asarray(x, dtype=np.float32)
    # pack per-core transposed fp16 inputs: [8, 128, HALF], j-major chunks
    xp = x.reshape(N_CORES, 2, HALF, D_IN)[:, :, idx, :].transpose(0, 1, 3, 2)
    xp = np.ascontiguousarray(xp, dtype=np.float16).reshape(N_CORES, 128, HALF)

    w1 = np.ascontiguousarray(
        np.concatenate([Wloc.T, Wloc.T], axis=0), dtype=np.float16)  # [128,128]
    w2 = np.ascontiguousarray((W / float(J)).T, dtype=np.float16)    # [128,10]

    in_maps = [{"xt": xp[c], "w1": w1, "w2": w2} for c in range(N_CORES)]
    res = run_bass_kernel_spmd(nc, in_maps, core_ids=list(range(N_CORES)))
    _CACHE["exec_time_ns"] = res.exec_time_ns
    _CACHE["trace"] = res.instructions_and_trace

    # host: sum the 4 PE column-group strips, then reorder segments
    out = np.empty((L // J, C), dtype=np.float32)
    for c in range(N_CORES):
        oc = res.results[c]["out"].astype(np.float32)  # [128, 5000] fp16
        strips = oc[0:10] + oc[32:42] + oc[64:74] + oc[96:106]  # [10, 5000]
        ocol = 0
        gbase = 0
        base = c * SEG_PER_CORE
        for tf in TFS:
            gt = tf // J
            blk = strips[:, ocol:ocol + 2 * gt].reshape(C, 2, gt)
            out[base + gbase:base + gbase + gt] = blk[:, 0].T
            out[base + HALF // J + gbase:base + HALF // J + gbase + gt] = blk[:, 1].T
            ocol += 2 * gt
            gbase += gt
    return out
